# revision 51
# baseline (speedup 1.0000x reference)
"""MoE BaseLayer kernel for 8 Trainium2 NeuronCores.

Strategy (expert-parallel, per the sharding hint):
  * Host computes the top-1 routing (argmax of x @ centroids.T), the sigmoid
    gate for the assigned expert, and the per-expert LayerNorm affine -- this
    IS the dispatch: tokens are gathered per-expert ("all-to-all" done
    host-side since kernel() receives full inputs) and each of the 8 cores
    gets one expert's tokens, padded to the max per-expert count C.
  * Each core runs the pure FFN for its tokens:
    relu(xn @ w1.T + b1) -> @ w2.T + b2 -> out = x + a * ffn   (bf16 matmuls,
    fp32 accumulation; everything in a D-major [D, tokens] layout so no
    on-device transposes are needed).
  * Host scatters per-expert outputs back to token order.

Device structure (two phases, PE never waits on DMA after warmup):
  * Phase 1 (mm1): for each token tile, psum[f_chunk(128), N] +=
    w1T[d_chunk, f_cols].T @ xn[d_chunk, N]; relu+b1 into h, which stays
    fully resident in SBUF ([128, 32, C] bf16, ~70KB/partition).
  * Phase 2 (mm2): d-chunk outer, so each w2 d-slab is streamed from HBM
    exactly once (8MB total), fully prefetched during phase 1:
    psum[d_chunk(128), N] += w2T[f_chunk, d_cols].T @ h[f_chunk, N],
    then (psum + b2) * gate + x, streamed out.
  * Gate a broadcast across partitions with rank-1 (K=1) matmuls against a
    ones row, in two bf16 pieces (hi+lo) to keep fp32 precision.
  * Host packs xn tile-major and x d-major so every DMA is a clean
    per-partition-contiguous transfer (no 512B strided descriptors).

fp8 was evaluated and rejected: DoubleRow fp8 matmul measures exactly 2x
bf16 FLOPs on HW, so any error-compensated scheme costs the same as bf16,
and plain fp8 e4m3 lands at rel_err ~3.2e-2 (> the 2e-2 gate).
"""

import sys

if "/opt/trn_rl_repo" not in sys.path:
    sys.path.insert(0, "/opt/trn_rl_repo")

import math

import ml_dtypes
import numpy as np

P = 128
D = 1024
F = 4096
E = 8
DC = D // P
FC = F // P
NCORES = 8
LN_EPS = 1e-5
BF16 = ml_dtypes.bfloat16

_compiled = {}


def _ensure_ntff_hook():
    """run_bass_kernel_spmd(trace=True) imports antenv.axon_hooks, which this
    container's antenv package lacks -- register the profiling hook via the
    libaxon_pjrt.so C ABI (mirrors trn_agent_boot.trn_boot) so tracing works
    instead of raising. No-op when the real module exists."""
    try:
        import antenv.axon_hooks  # noqa: F401

        return
    except ImportError:
        pass
    import contextlib
    import ctypes
    import types

    try:
        lib = ctypes.CDLL("/opt/axon/libaxon_pjrt.so")
        if not hasattr(lib, "axon_start_nrt_profile"):
            raise OSError("no profile ABI")
        lib.axon_start_nrt_profile.argtypes = [
            ctypes.POINTER(ctypes.c_int64),
            ctypes.c_size_t,
        ]
        lib.axon_start_nrt_profile.restype = ctypes.c_int64
        lib.axon_stop_nrt_profile.argtypes = [ctypes.c_char_p]
        lib.axon_stop_nrt_profile.restype = ctypes.c_int64

        @contextlib.contextmanager
        def _hook(output_dir, device_ids):
            import jax

            jax.devices()
            if device_ids:
                ids = (ctypes.c_int64 * len(device_ids))(*device_ids)
                rc = lib.axon_start_nrt_profile(ids, len(device_ids))
            else:
                rc = lib.axon_start_nrt_profile(None, 0)
            if rc != 0:
                raise RuntimeError(f"axon_start_nrt_profile rc={rc}")
            try:
                yield
            finally:
                lib.axon_stop_nrt_profile(str(output_dir).encode())

        get_hook = lambda: _hook  # noqa: E731
    except OSError:
        get_hook = lambda: None  # noqa: E731

    mod = types.ModuleType("antenv.axon_hooks")
    mod.get_axon_ntff_profile_hook = get_hook
    mod.set_axon_ntff_profile_hook = lambda h: None
    sys.modules["antenv.axon_hooks"] = mod
    try:
        import antenv

        antenv.axon_hooks = mod
    except ImportError:
        pass


def _token_tiles(C):
    """Token tile sizes <= 512: a small first tile so the first mm1 starts
    as soon as its xn slice + the first w1 group land, the rest balanced."""
    if C <= 512:
        return [(0, C)]
    first = 256
    rest = C - first
    nt = max(1, math.ceil(rest / 512))
    base = rest // nt
    rem = rest % nt
    sizes = [first] + [base + (1 if i < rem else 0) for i in range(nt)]
    tiles = []
    s = 0
    for n in sizes:
        tiles.append((s, n))
        s += n
    return tiles


def _build(C, segs=()):
    """segs: overflow-token segment sizes (tokens of experts whose count
    exceeded the per-core cap). Every core runs the same extra-section code
    over these tokens but contracts a different F-slice (F/8 hidden units)
    of the owning expert's weights -- the slice lives in the per-core weight
    DATA, so the NEFF stays uniform. Partials are summed on the host."""
    import concourse.tile as tile
    from concourse import bacc, mybir

    f32 = mybir.dt.float32
    bf16 = mybir.dt.bfloat16
    AF = mybir.ActivationFunctionType

    tiles = _token_tiles(C)
    NMAX = max(n for _, n in tiles)
    SEG = len(segs)
    NE = sum(segs)
    F2 = (F // 8) // P  # f-chunks per core's slice of the extra section

    nc = bacc.Bacc("TRN2", target_bir_lowering=False, debug=False)

    FG = F // 8  # w1 f-column group size
    # xnp: tile-major packed layernormed tokens [P, (tile)(c)(n)]
    xnp = nc.dram_tensor("xnp", (P, DC * C), bf16, kind="ExternalInput").ap()
    # xp: d-major packed residual [P, (d)(n)]
    xp = nc.dram_tensor("xp", (P, DC * C), bf16, kind="ExternalInput").ap()
    # gate hi/lo: bf16 pieces summing to ~fp32 sigmoid alpha (split on host)
    gh = nc.dram_tensor("gh", (1, C), bf16, kind="ExternalInput").ap()
    gl = nc.dram_tensor("gl", (1, C), bf16, kind="ExternalInput").ap()
    w1r = nc.dram_tensor("w1r", (8, P, DC * FG), bf16, kind="ExternalInput").ap()
    w2s = nc.dram_tensor("w2s", (DC, P, FC * P), bf16, kind="ExternalInput").ap()
    b1r = nc.dram_tensor("b1r", (P, FC), f32, kind="ExternalInput").ap()
    b2r = nc.dram_tensor("b2r", (P, DC), f32, kind="ExternalInput").ap()
    outT = nc.dram_tensor("outT", (D, C), f32, kind="ExternalOutput").ap()
    if SEG:
        # extra section: xn of overflow tokens (seg-major [ (s)(c)(n) ]),
        # this core's F-slice of w1/w2/b1, and the raw partial-y output
        xen = nc.dram_tensor("xen", (P, DC * NE), bf16, kind="ExternalInput").ap()
        w1x = nc.dram_tensor("w1x", (P, SEG * DC * F2 * P), bf16, kind="ExternalInput").ap()
        w2x = nc.dram_tensor("w2x", (P, SEG * F2 * DC * P), bf16, kind="ExternalInput").ap()
        b1x = nc.dram_tensor("b1x", (P, SEG * F2), f32, kind="ExternalInput").ap()
        outx = nc.dram_tensor("outx", (P, DC * NE), f32, kind="ExternalOutput").ap()

    ov = outT.rearrange("(c p) n -> p c n", p=P)
    w1v = w1r.rearrange("g p (c j) -> g p c j", c=DC)
    xpv = xp.rearrange("p (c n) -> p c n", c=DC)

    with tile.TileContext(nc) as tc:
        with (
            tc.tile_pool(name="wres", bufs=1) as wres,
            tc.tile_pool(name="w2p", bufs=2) as w2p,
            tc.tile_pool(name="wxp", bufs=2) as wxp,
            tc.tile_pool(name="cst", bufs=1) as cst,
            tc.tile_pool(name="big", bufs=1) as big,
            tc.tile_pool(name="xdp", bufs=2) as xdp,
            tc.tile_pool(name="ctp", bufs=2) as ctp,
            tc.tile_pool(name="otp", bufs=2) as otp,
            tc.tile_pool(name="prep", bufs=2, space="PSUM") as prep,
            tc.tile_pool(name="php", bufs=2, space="PSUM") as php,
            tc.tile_pool(name="pyp", bufs=2, space="PSUM") as pyp,
        ):
            # ---- DMA ordering on the SP ring: tiny constants first (the PE
            # queue hits the gate broadcast early), then tile 0's xn, then
            # ALL w1 groups (mm1 tile 0 consumes them at ~3.5us per group --
            # streaming stays just ahead), then the later xn tiles; w2 slabs
            # and per-d x slices are prefetched during phase 1.
            ah_sb = cst.tile([1, C], bf16)
            nc.sync.dma_start(ah_sb[:], gh)
            al_sb = cst.tile([1, C], bf16)
            nc.sync.dma_start(al_sb[:], gl)
            b1_sb = cst.tile([P, FC], f32)
            nc.sync.dma_start(b1_sb[:], b1r)
            b2_sb = cst.tile([P, DC], f32)
            nc.sync.dma_start(b2_sb[:], b2r)

            # tile 0's xn and half of the first w1 group ride the
            # (startup-idle) ACT ring, in parallel with the SP ring, so mm1
            # starts a few us earlier
            xn_sb = big.tile([P, DC, C], bf16)
            xn_tiles = []
            off = 0
            for ti, (S, N) in enumerate(tiles):
                xt = xn_sb[:, :, S : S + N]
                xn_tiles.append(xt)
                if ti == 0:
                    nc.scalar.dma_start(
                        xt, xnp[:, off : off + DC * N].rearrange("p (c n) -> p c n", c=DC)
                    )
                off += DC * N

            w1g = []
            for fg in range(8):
                wt = wres.tile([P, DC, FG], bf16, name=f"w1g{fg}")
                if fg == 0:
                    nc.scalar.dma_start(wt[:, 0 : DC // 2], w1v[fg][:, 0 : DC // 2])
                    nc.sync.dma_start(wt[:, DC // 2 :], w1v[fg][:, DC // 2 :])
                else:
                    nc.sync.dma_start(wt[:], w1v[fg])
                w1g.append(wt)

            off = DC * tiles[0][1]
            for ti, (S, N) in enumerate(tiles):
                if ti > 0:
                    nc.sync.dma_start(
                        xn_tiles[ti],
                        xnp[:, off : off + DC * N].rearrange("p (c n) -> p c n", c=DC),
                    )
                    off += DC * N

            if SEG:
                seg_off = []
                _o = 0
                for n in segs:
                    seg_off.append(_o)
                    _o += n
                xen_sb = cst.tile([P, DC * NE], bf16)
                nc.sync.dma_start(xen_sb[:], xen)
                b1x_sb = cst.tile([P, SEG * F2], f32)
                nc.sync.dma_start(b1x_sb[:], b1x)
                he_sb = cst.tile([P, SEG * F2, max(segs)], bf16)
                w1xv = w1x.rearrange("p (s c j) -> p s c j", s=SEG, c=DC)
                w2xv = w2x.rearrange("p (s f j) -> p s f j", s=SEG, f=F2)

            ones_row_bf = cst.tile([1, P], bf16)
            nc.vector.memset(ones_row_bf[:], 1.0)
            ones_col = cst.tile([P, 1], bf16)
            nc.vector.memset(ones_col[:], 1.0)

            # HAM warmup: cheap bf16 dummy matmuls during the initial DMA
            # wait, so the PE clock gate is open when the real work starts.
            WN = 256
            scr_bf = cst.tile([P, WN], bf16)
            nc.vector.memset(scr_bf[:], 0.0)
            psw = prep.tile([P, NMAX], f32, tag="rep", name="psw")[0:1, :WN]
            for _ in range(32):
                nc.tensor.matmul(psw, ones_col[:], scr_bf[:], start=True, stop=True)

            repa_sb = big.tile([P, C], f32)
            h_sb = big.tile([P, FC, C], bf16)
            w2_queue = []
            x_queue = []

            # ---- phase 1: mm1 + relu per token tile ----
            for ti, (S, N) in enumerate(tiles):
                sl = slice(S, S + N)
                for f in range(FC):
                    ph = php.tile([P, NMAX], f32, tag="ph", name="ph")[:, :N]
                    wg = w1g[f // 4]
                    fo = f % 4
                    for c in range(DC):
                        nc.tensor.matmul(
                            ph,
                            wg[:, c, fo * P : (fo + 1) * P],
                            xn_sb[:, c, sl],
                            start=(c == 0),
                            stop=(c == DC - 1),
                        )
                    nc.scalar.activation(
                        h_sb[:, f, sl], ph, AF.Relu, bias=b1_sb[:, f : f + 1]
                    )
                # prefetch phase-2 streams while mm1 owns the PE
                if ti == 0:
                    for d in range(2):
                        w2t = w2p.tile([P, FC * P], bf16, tag="w2", name=f"w2_{d}")
                        nc.sync.dma_start(w2t[:], w2s[d])
                        w2_queue.append(w2t)
                        xd = xdp.tile([P, C], bf16, tag="xd", name=f"xd{d}")
                        nc.sync.dma_start(xd[:], xpv[:, d, :])
                        x_queue.append(xd)

            # ---- extra section: overflow tokens, this core's F-slice.
            # Emitted as small jobs interleaved between the main phase-2
            # blocks so their chain latency hides under the big matmuls.
            ex_jobs = []
            if SEG:
                wx_tiles = {}

                def dma_wx(kind, s):
                    def go():
                        if kind == "w1":
                            wx = wxp.tile(
                                [P, DC, F2 * P], bf16, tag="wx", name=f"w1x{s}"
                            )
                            nc.sync.dma_start(wx[:], w1xv[:, s])
                        else:
                            wx = wxp.tile(
                                [P, F2, DC * P], bf16, tag="wx", name=f"w2x{s}"
                            )
                            nc.sync.dma_start(wx[:], w2xv[:, s])
                        wx_tiles[(kind, s)] = wx

                    return go

                def ex1_job(s, f):
                    def go():
                        n = segs[s]
                        xb = DC * seg_off[s]
                        ph = php.tile([P, NMAX], f32, tag="ph", name="ph")[:, :n]
                        for c in range(DC):
                            nc.tensor.matmul(
                                ph,
                                wx_tiles[("w1", s)][:, c, f * P : (f + 1) * P],
                                xen_sb[:, xb + c * n : xb + (c + 1) * n],
                                start=(c == 0),
                                stop=(c == DC - 1),
                            )
                        sf = s * F2 + f
                        nc.scalar.activation(
                            he_sb[:, sf, :n], ph, AF.Relu, bias=b1x_sb[:, sf : sf + 1]
                        )

                    return go

                def ex2_job(s, d):
                    def go():
                        n = segs[s]
                        py = php.tile([P, NMAX], f32, tag="ph", name="ph")[:, :n]
                        for f in range(F2):
                            nc.tensor.matmul(
                                py,
                                wx_tiles[("w2", s)][:, f, d * P : (d + 1) * P],
                                he_sb[:, s * F2 + f, :n],
                                start=(f == 0),
                                stop=(f == F2 - 1),
                            )
                        ox = ctp.tile([P, NMAX], f32, tag="ct", name="ct")[:, :n]
                        nc.scalar.copy(ox, py)
                        ob = DC * seg_off[s] + d * n
                        nc.gpsimd.dma_start(outx[:, ob : ob + n], ox)

                    return go

                # wx slab DMAs sequenced so each lands only after the slab
                # whose pool buffer it reuses (2 tiles back) has been consumed
                groups = [("ex1", s) for s in range(SEG)] + [
                    ("ex2", s) for s in range(SEG)
                ]
                tiles_seq = [("w1", s) for s in range(SEG)] + [
                    ("w2", s) for s in range(SEG)
                ]
                post = {g: [] for g in range(len(groups))}
                for k, (kind, s) in enumerate(tiles_seq):
                    job = dma_wx(kind, s)
                    if k < 2:
                        ex_jobs.append(job)
                    else:
                        post[k - 2].append(job)
                for g, (kind, s) in enumerate(groups):
                    if kind == "ex1":
                        ex_jobs += [ex1_job(s, f) for f in range(F2)]
                    else:
                        ex_jobs += [ex2_job(s, d) for d in range(DC)]
                    ex_jobs += post[g]

            # gate broadcasts (cheap rank-1 matmuls) ahead of the epilogues
            for ti, (S, N) in enumerate(tiles):
                sl = slice(S, S + N)
                ra = prep.tile([P, NMAX], f32, tag="rep", name="rep")[:, :N]
                nc.tensor.matmul(ra, ones_row_bf[:], ah_sb[:, sl], start=True, stop=False)
                nc.tensor.matmul(ra, ones_row_bf[:], al_sb[:, sl], start=False, stop=True)
                nc.scalar.copy(repa_sb[:, sl], ra)

            # ---- phase 2: mm2 + epilogue, d-chunk outer (w2 streamed once) ----
            nb = DC * len(tiles)
            emit_blocks = max(1, nb - 2)
            ex_emitted = 0
            blk = 0
            for d in range(DC):
                w2t = w2_queue.pop(0)
                xd = x_queue.pop(0)
                if d + 2 < DC:
                    nw = w2p.tile([P, FC * P], bf16, tag="w2", name=f"w2_{d + 2}")
                    nc.sync.dma_start(nw[:], w2s[d + 2])
                    w2_queue.append(nw)
                    nx = xdp.tile([P, C], bf16, tag="xd", name=f"xd{d + 2}")
                    nc.sync.dma_start(nx[:], xpv[:, d + 2, :])
                    x_queue.append(nx)
                last_d = d == DC - 1
                for ti, (S, N) in enumerate(tiles):
                    # split the very last (d, tile) so its combine/store
                    # overlaps the final matmuls instead of trailing
                    if last_d and ti == len(tiles) - 1:
                        q = N // 4
                        halves = [(0, q), (q, q), (2 * q, q), (3 * q, N - 3 * q)]
                    else:
                        halves = [(0, N)]
                    for h0, hn in halves:
                        hsl = slice(S + h0, S + h0 + hn)
                        py = pyp.tile([P, NMAX], f32, tag="py", name="py")[:, :hn]
                        for fi in range(FC):
                            nc.tensor.matmul(
                                py,
                                w2t[:, fi * P : (fi + 1) * P],
                                h_sb[:, fi, hsl],
                                start=(fi == 0),
                                stop=(fi == FC - 1),
                            )
                        tcm = ctp.tile([P, NMAX], f32, tag="ct", name="ct")[:, :hn]
                        nc.scalar.add(tcm, py, b2_sb[:, d : d + 1])
                        nc.vector.tensor_mul(tcm, tcm, repa_sb[:, hsl])
                        ot = otp.tile([P, NMAX], f32, tag="ot", name="ot")[:, :hn]
                        nc.vector.tensor_add(ot, tcm, xd[:, hsl])
                        if last_d:
                            nc.sync.dma_start(ov[:, d, hsl], ot)
                        else:
                            nc.gpsimd.dma_start(ov[:, d, hsl], ot)
                    blk += 1
                    if blk <= emit_blocks and ex_jobs:
                        quota = (len(ex_jobs) * blk) // emit_blocks
                        while ex_emitted < quota:
                            ex_jobs[ex_emitted]()
                            ex_emitted += 1

    nc.compile()
    return nc


def _get_compiled(C, segs=()):
    key = (C, tuple(segs))
    if key not in _compiled:
        _compiled[key] = _build(C, tuple(segs))
    return _compiled[key]


def _prep(inputs):
    x = np.ascontiguousarray(
        np.asarray(inputs["input_features"], np.float32).reshape(-1, D)
    )
    T = x.shape[0]
    cent = np.asarray(inputs["centroids"], np.float64)
    w1 = np.asarray(inputs["w1"], np.float32)
    b1 = np.asarray(inputs["b1"], np.float32)
    w2 = np.asarray(inputs["w2"], np.float32)
    b2 = np.asarray(inputs["b2"], np.float32)
    ln_g = np.asarray(inputs["ln_g"], np.float64)
    ln_b = np.asarray(inputs["ln_b"], np.float64)

    xd = x.astype(np.float64)
    aff = xd @ cent.T
    assign = aff.argmax(1)
    alpha = 1.0 / (1.0 + np.exp(-aff[np.arange(T), assign]))

    # host-side LayerNorm (the dispatch side of the layer): xhat in f64
    mu = xd.mean(1, keepdims=True)
    var = xd.var(1, keepdims=True)
    xhat = (xd - mu) / np.sqrt(var + LN_EPS)

    counts = np.bincount(assign, minlength=E)
    cap = (T + NCORES - 1) // NCORES
    if int(counts.max()) > cap:
        # balance: cap the per-core main section; the overflow tokens run in
        # a uniform extra section where each core contracts a different
        # F-slice (host sums the partials)
        C = cap
    else:
        C = max(int(counts.max()), P)
    tiles = _token_tiles(C)

    def xn_rows(ix, e):
        return (xhat[ix] * ln_g[e][None, :] + ln_b[e][None, :]).astype(BF16)

    seg_list = []  # (expert, token-idx array), chunks <= 512
    for e in range(E):
        idx_full = np.nonzero(assign == e)[0]
        if len(idx_full) > C:
            ov = idx_full[C:]
            for s0 in range(0, len(ov), 512):
                seg_list.append((e, ov[s0 : s0 + 512]))
    segs = tuple(len(ix) for _, ix in seg_list)
    SEG, NE = len(segs), sum(segs)
    FS = F // NCORES
    F2 = FS // P

    if SEG:
        xen_pk = np.empty((P, DC * NE), BF16)
        off = 0
        for e, ix in seg_list:
            n = len(ix)
            blk = xn_rows(ix, e).reshape(n, DC, P).transpose(2, 1, 0)
            xen_pk[:, off : off + DC * n] = blk.reshape(P, DC * n)
            off += DC * n

    idx_list = []
    in_maps = []
    for e in range(NCORES):
        idx = np.nonzero(assign == e)[0][:C]
        cnt = len(idx)
        idx_list.append(idx)

        # per-expert LN affine folded on the host: xn = xhat * g_e + b_e
        xn = np.zeros((C, D), BF16)
        xn[:cnt] = xn_rows(idx, e)
        # [C, D] -> per-tile [P, DC, N] packed -> [P, DC*C]
        xnc = xn.reshape(C, DC, P)
        xnpk = np.empty((P, DC * C), BF16)
        off = 0
        for S, N in tiles:
            blk = xnc[S : S + N].transpose(2, 1, 0)  # [P, DC, N]
            xnpk[:, off : off + DC * N] = blk.reshape(P, DC * N)
            off += DC * N

        xb = np.zeros((C, D), BF16)
        xb[:cnt] = x[idx].astype(BF16)
        # [C, D] -> d-major [P, DC*C]
        xpk = np.ascontiguousarray(
            xb.reshape(C, DC, P).transpose(2, 1, 0).reshape(P, DC * C)
        )

        a_e = np.zeros(C, np.float32)
        a_e[:cnt] = alpha[idx]
        a_hi = a_e.astype(BF16)
        a_lo = (a_e - a_hi.astype(np.float32)).astype(BF16)

        w1T = w1[e].T  # [D, F]
        FG = F // 8
        w1rb = np.ascontiguousarray(
            w1T.reshape(DC, P, 8, FG).transpose(2, 1, 0, 3).reshape(8, P, DC * FG)
        ).astype(BF16)
        w2T = w2[e].T  # [F, D]
        w2sb = np.ascontiguousarray(
            w2T.reshape(FC, P, DC, P).transpose(2, 1, 0, 3).reshape(DC, P, FC * P)
        ).astype(BF16)

        im = {
            "xnp": xnpk,
            "xp": xpk,
            "gh": a_hi.reshape(1, C),
            "gl": a_lo.reshape(1, C),
            "w1r": w1rb,
            "w2s": w2sb,
            "b1r": np.ascontiguousarray(b1[e].reshape(FC, P).T),
            "b2r": np.ascontiguousarray(b2[e].reshape(DC, P).T),
        }
        if SEG:
            fs = slice(FS * e, FS * (e + 1))
            w1xp = np.empty((P, SEG, DC, F2 * P), BF16)
            w2xp = np.empty((P, SEG, F2, DC * P), BF16)
            b1xp = np.empty((P, SEG * F2), np.float32)
            for s, (ee, _ix) in enumerate(seg_list):
                t = w1[ee][fs].T.reshape(DC, P, F2, P)  # [c, p_d, f, j_f]
                w1xp[:, s] = t.transpose(1, 0, 2, 3).reshape(P, DC, F2 * P)
                t2 = w2[ee][:, fs].T.reshape(F2, P, DC, P)  # [f, p_f, d, j_d]
                w2xp[:, s] = t2.transpose(1, 0, 2, 3).reshape(P, F2, DC * P)
                b1xp[:, s * F2 : (s + 1) * F2] = b1[ee][fs].reshape(F2, P).T
            im["xen"] = xen_pk
            im["w1x"] = np.ascontiguousarray(w1xp.reshape(P, -1))
            im["w2x"] = np.ascontiguousarray(w2xp.reshape(P, -1))
            im["b1x"] = b1xp
        in_maps.append(im)

    def combine(results, out):
        """sum the per-core F-slice partials and finish the overflow tokens
        (bias, gate, residual) on the host"""
        if not SEG:
            return
        ysum = np.zeros((P, DC * NE), np.float32)
        for k in range(NCORES):
            ysum += results[k]["outx"]
        off = 0
        for e, ix in seg_list:
            n = len(ix)
            blk = ysum[:, off : off + DC * n].reshape(P, DC, n)
            y = blk.transpose(1, 0, 2).reshape(D, n).T + b2[e]  # [n, D]
            out[ix] = x[ix] + alpha[ix, None].astype(np.float32) * y
            off += DC * n

    return C, segs, idx_list, in_maps, T, combine


def _run(inputs, trace=False, trace_cores=None, stitch_traces=False):
    _ensure_ntff_hook()
    from concourse.bass_utils import run_bass_kernel_spmd

    C, segs, idx_list, in_maps, T, combine = _prep(inputs)
    nc = _get_compiled(C, segs)
    res = run_bass_kernel_spmd(
        nc,
        in_maps,
        core_ids=list(range(NCORES)),
        trace=trace,
        trace_cores=trace_cores,
        stitch_traces=stitch_traces,
    )
    out = np.zeros((T, D), np.float32)
    for e in range(NCORES):
        idx = idx_list[e]
        out[idx] = res.results[e]["outT"][:, : len(idx)].T
    combine(res.results, out)
    out = out.reshape(np.asarray(inputs["input_features"]).shape)
    return out, res


def kernel(**inputs):
    out, _ = _run(inputs)
    return out
